# revision 3
# baseline (speedup 1.0000x reference)
"""Expert-parallel MoE GLU MLP kernel for Trainium2.

8 experts -> 8 NeuronCores, one expert per core (no collectives needed).
Per core:  x (C,H) @ w_gate_up (H,2I) -> GLU -> (C,I) @ w_down (I,H) -> (C,H)

Layout strategy (per core):
  - x is PE-transposed on chip to xT (H on partitions) once: 8 MB SBUF resident.
  - GEMM1: stationary = w_gate_up column slices [128h x 128f], moving = xT
    -> psum (f, c); GLU (silu(gate)*up) fused on ACT+DVE -> act in (I, C)
    layout, round-tripped through DRAM (keeps SBUF pressure low).
  - GEMM2: stationary = act tiles [128i x 128c], moving = w_down rows
    -> out (C, H) accumulated in an 8 MB SBUF accumulator over I-blocks.
  - All matmuls run as float32r (full-rate fp32 mode at free dim >= 256).
"""
import numpy as np

E, C, H, I = 8, 1024, 2048, 4096
P = 128
HT, IT, CT = H // P, I // P, C // P  # 16, 32, 8

_CACHE = {}


def _build():
    import concourse.bacc as bacc
    import concourse.mybir as mybir
    import concourse.tile as tile
    from concourse.masks import make_identity

    f32 = mybir.dt.float32
    f32r = mybir.dt.float32r
    AF = mybir.ActivationFunctionType

    nc = bacc.Bacc("TRN2", target_bir_lowering=False, debug=False)
    # wgu/wdn feed f32r matmuls straight from DMA: declare them f32r so the
    # producer/consumer dtype chain is consistent (np view is float32 either
    # way). x feeds the f32 PE-transpose path, out is plain f32.
    x = nc.declare_dram_parameter("x", [C, H], f32, isOutput=False).ap()
    wgu = nc.declare_dram_parameter("wgu", [H, 2 * I], f32r, isOutput=False).ap()
    wdn = nc.declare_dram_parameter("wdn", [I, H], f32r, isOutput=False).ap()
    out = nc.declare_dram_parameter("out", [C, H], f32, isOutput=True).ap()

    wgu_v = wgu.rearrange("(ht p) f -> p ht f", p=P)  # [128, 16, 8192]
    wdn_v = wdn.rearrange("(it p) h -> p it h", p=P)  # [128, 32, 2048]
    out_v = out.rearrange("(ct p) h -> p ct h", p=P)  # [128, 8, 2048]

    with tile.TileContext(nc) as tc:
        with tc.tile_pool(name="dram", bufs=1, space="DRAM") as dram:
            # act_d[p, it, c] = act row (it*128+p), col c
            act_d = dram.tile([P, IT, C], f32r)

            with tc.tile_pool(name="xt_pool", bufs=1) as xtp:
                # xt[p, ht, c] = x[c, ht*128+p] (f32r: the copy below rounds)
                xt = xtp.tile([P, HT, C], f32r)

                # ---- Phase 0: transpose x into SBUF ----------------------
                with (
                    tc.tile_pool(name="constp", bufs=1) as constp,
                    tc.tile_pool(name="xload", bufs=2) as xl,
                    tc.tile_pool(name="tps", bufs=4, space="PSUM") as tps,
                ):
                    ident = constp.tile([P, P], f32)
                    make_identity(nc, ident)
                    for ct in range(CT):
                        xrow = xl.tile([P, H], f32, tag="xrow")
                        nc.sync.dma_start(xrow, x[ct * P:(ct + 1) * P, :])
                        for ht in range(HT):
                            tp = tps.tile([P, P], f32, tag="tp")
                            nc.tensor.transpose(
                                tp, xrow[:, ht * P:(ht + 1) * P], ident)
                            nc.vector.tensor_copy(
                                xt[:, ht, ct * P:(ct + 1) * P], tp)

                # ---- Phase 1: gate_up GEMM + GLU -> act_d ----------------
                with (
                    tc.tile_pool(name="w1", bufs=2) as w1,
                    tc.tile_pool(name="sb1", bufs=3) as sb1,
                    tc.tile_pool(name="ps1", bufs=2, space="PSUM") as ps1,
                ):
                    for i in range(IT):
                        wg = w1.tile([P, HT, P], f32r, tag="wg")
                        nc.sync.dma_start(wg, wgu_v[:, :, i * P:(i + 1) * P])
                        wu = w1.tile([P, HT, P], f32r, tag="wu")
                        nc.sync.dma_start(
                            wu, wgu_v[:, :, I + i * P:I + (i + 1) * P])
                        for cc in range(2):
                            cs = slice(cc * 512, (cc + 1) * 512)
                            pg = ps1.tile([P, 512], f32, tag="pg")
                            pu = ps1.tile([P, 512], f32, tag="pu")
                            for ht in range(HT):
                                nc.tensor.matmul(
                                    pg, wg[:, ht, :], xt[:, ht, cs],
                                    start=(ht == 0), stop=(ht == HT - 1))
                            for ht in range(HT):
                                nc.tensor.matmul(
                                    pu, wu[:, ht, :], xt[:, ht, cs],
                                    start=(ht == 0), stop=(ht == HT - 1))
                            sil = sb1.tile([P, 512], f32, tag="sil")
                            nc.scalar.activation(sil, pg, AF.Silu)
                            av = sb1.tile([P, 512], f32r, tag="av")
                            nc.vector.tensor_mul(av, sil, pu)
                            nc.sync.dma_start(act_d[:, i, cs], av)

            # ---- Phase 2: down GEMM, SBUF accumulator over I-blocks ------
            with (
                tc.tile_pool(name="oaccp", bufs=1) as op_,
                tc.tile_pool(name="w2", bufs=2) as w2,
                tc.tile_pool(name="a2", bufs=2) as a2,
                tc.tile_pool(name="ps2", bufs=4, space="PSUM") as ps2,
            ):
                oacc = op_.tile([P, CT, H], f32)  # out[ct*128+p, h]
                NB = 4  # i-tiles per block
                for ib in range(IT // NB):
                    wdb = w2.tile([P, NB, H], f32r, tag="wdb")
                    nc.sync.dma_start(wdb, wdn_v[:, ib * NB:(ib + 1) * NB, :])
                    ab = a2.tile([P, NB, C], f32r, tag="ab")
                    nc.sync.dma_start(ab, act_d[:, ib * NB:(ib + 1) * NB, :])
                    for ct in range(CT):
                        for hc in range(4):
                            hs = slice(hc * 512, (hc + 1) * 512)
                            ps = ps2.tile([P, 512], f32, tag="ps")
                            for ii in range(NB):
                                nc.tensor.matmul(
                                    ps,
                                    ab[:, ii, ct * P:(ct + 1) * P],
                                    wdb[:, ii, hs],
                                    start=(ii == 0), stop=(ii == NB - 1))
                            if ib == 0:
                                nc.vector.tensor_copy(oacc[:, ct, hs], ps)
                            else:
                                nc.vector.tensor_add(
                                    oacc[:, ct, hs], oacc[:, ct, hs], ps)
                nc.sync.dma_start(out_v, oacc)

    nc.compile()
    return nc


def _get_nc():
    if "nc" not in _CACHE:
        _CACHE["nc"] = _build()
    return _CACHE["nc"]


def _run(hidden_states, w_gate_up, w_down, trace=False):
    from concourse.bass_utils import run_bass_kernel_spmd

    nc = _get_nc()
    hs = np.ascontiguousarray(np.asarray(hidden_states, dtype=np.float32))
    wg = np.ascontiguousarray(np.asarray(w_gate_up, dtype=np.float32))
    wd = np.ascontiguousarray(np.asarray(w_down, dtype=np.float32))
    in_maps = [
        {"x": hs[e], "wgu": wg[e], "wdn": wd[e]} for e in range(E)
    ]
    res = run_bass_kernel_spmd(nc, in_maps, list(range(E)), trace=trace)
    output = np.stack([res.results[e]["out"] for e in range(E)], axis=0)
    return output, res


def kernel(hidden_states, w_gate_up, w_down):
    output, _ = _run(hidden_states, w_gate_up, w_down, trace=False)
    return output


# revision 6
# speedup vs baseline: 1.0779x; 1.0779x over previous
"""Expert-parallel MoE GLU MLP kernel for Trainium2.

8 experts -> 8 NeuronCores, one expert per core (no collectives needed).
Per core:  x (C,H) @ w_gate_up (H,2I) -> GLU -> (C,I) @ w_down (I,H) -> (C,H)

Strategy (per core):
  - Host passes x pre-transposed (H,C) per expert; it lands in SBUF as the
    f32r moving operand of GEMM1 (8 MB resident).
  - GEMM1 (f32r, full-rate fp32 mode): stationary = w_gate_up column slices
    [128h x 128f], moving = xT -> psum (f, c). GLU = silu(gate) [ACT] * up
    [DVE] writes straight into a bf16 SBUF-resident act tile (I, C). No DRAM
    round-trip for activations.
  - GEMM2 (bf16): stationary = act tiles [128i x 128c], moving = w_down rows
    cast f32->bf16 during SWDGE DMA -> psum accumulates the full I chain
    -> one copy -> out (C,H). No SBUF accumulator adds.
"""
import numpy as np

E, C, H, I = 8, 1024, 2048, 4096
P = 128
HT, IT, CT = H // P, I // P, C // P  # 16, 32, 8

_CACHE = {}


def _build():
    import concourse.bacc as bacc
    import concourse.mybir as mybir
    import concourse.tile as tile

    f32 = mybir.dt.float32
    f32r = mybir.dt.float32r
    bf16 = mybir.dt.bfloat16
    AF = mybir.ActivationFunctionType

    nc = bacc.Bacc("TRN2", target_bir_lowering=False, debug=False)
    # xT/wgu feed f32r matmuls straight from DMA: declare them f32r so the
    # producer/consumer dtype chain is consistent (np view is float32 either
    # way). wdn is cast f32->bf16 during its SWDGE load. out is plain f32.
    xT = nc.declare_dram_parameter("xT", [H, C], f32r, isOutput=False).ap()
    wgu = nc.declare_dram_parameter("wgu", [H, 2 * I], f32r, isOutput=False).ap()
    wdn = nc.declare_dram_parameter("wdn", [I, H], f32, isOutput=False).ap()
    out = nc.declare_dram_parameter("out", [C, H], f32, isOutput=True).ap()

    xT_v = xT.rearrange("(ht p) c -> p ht c", p=P)    # [128, 16, 1024]
    wgu_v = wgu.rearrange("(ht p) f -> p ht f", p=P)  # [128, 16, 8192]
    wdn_v = wdn.rearrange("(it p) h -> p it h", p=P)  # [128, 32, 2048]
    out_v = out.rearrange("(ct p) h -> p ct h", p=P)  # [128, 8, 2048]

    with tile.TileContext(nc) as tc:
        with tc.tile_pool(name="acts_pool", bufs=1) as actsp:
            # acts[p, it, c] = act row (it*128+p), col c  (bf16, 8 MB)
            acts = actsp.tile([P, IT, C], bf16)

            # ---- Phase 1: gate_up GEMM (f32r) + GLU -> acts --------------
            with (
                tc.tile_pool(name="xt_pool", bufs=1) as xtp,
                tc.tile_pool(name="w1", bufs=2) as w1,
                tc.tile_pool(name="sb1", bufs=3) as sb1,
                tc.tile_pool(name="ps1", bufs=2, space="PSUM") as ps1,
            ):
                xt = xtp.tile([P, HT, C], f32r)  # xt[p, ht, c] = x[c, ht*128+p]
                nc.sync.dma_start(xt, xT_v)
                for i in range(IT):
                    wg = w1.tile([P, HT, P], f32r, tag="wg")
                    nc.sync.dma_start(wg, wgu_v[:, :, i * P:(i + 1) * P])
                    wu = w1.tile([P, HT, P], f32r, tag="wu")
                    nc.sync.dma_start(
                        wu, wgu_v[:, :, I + i * P:I + (i + 1) * P])
                    for cc in range(2):
                        cs = slice(cc * 512, (cc + 1) * 512)
                        pg = ps1.tile([P, 512], f32, tag="pg")
                        pu = ps1.tile([P, 512], f32, tag="pu")
                        for ht in range(HT):
                            nc.tensor.matmul(
                                pg, wg[:, ht, :], xt[:, ht, cs],
                                start=(ht == 0), stop=(ht == HT - 1))
                        for ht in range(HT):
                            nc.tensor.matmul(
                                pu, wu[:, ht, :], xt[:, ht, cs],
                                start=(ht == 0), stop=(ht == HT - 1))
                        sil = sb1.tile([P, 512], f32, tag="sil")
                        nc.scalar.activation(sil, pg, AF.Silu)
                        nc.vector.tensor_mul(acts[:, i, cs], sil, pu)

        # ---- Phase 2: down GEMM (bf16), full-I psum chains ---------------
            with (
                tc.tile_pool(name="w2", bufs=2) as w2,
                tc.tile_pool(name="sb2", bufs=3) as sb2,
                tc.tile_pool(name="ps2", bufs=4, space="PSUM") as ps2,
            ):
                NHC = 8  # h-chunks of 256 cols
                HW_ = H // NHC
                for hc in range(NHC):
                    hs = slice(hc * HW_, (hc + 1) * HW_)
                    wsf = w2.tile([P, IT, HW_], f32, tag="wsf")
                    nc.sync.dma_start(wsf, wdn_v[:, :, hs])
                    wsl = w2.tile([P, IT, HW_], bf16, tag="wsl")
                    nc.vector.tensor_copy(wsl, wsf)
                    for ct in range(CT):
                        ps = ps2.tile([P, HW_], f32, tag="ps")
                        for i in range(IT):
                            nc.tensor.matmul(
                                ps,
                                acts[:, i, ct * P:(ct + 1) * P],
                                wsl[:, i, :],
                                start=(i == 0), stop=(i == IT - 1))
                        osb = sb2.tile([P, HW_], f32, tag="osb")
                        nc.vector.tensor_copy(osb, ps)
                        nc.sync.dma_start(out_v[:, ct, hs], osb)

    nc.compile()
    return nc


def _get_nc():
    if "nc" not in _CACHE:
        _CACHE["nc"] = _build()
    return _CACHE["nc"]


def _run(hidden_states, w_gate_up, w_down, trace=False):
    from concourse.bass_utils import run_bass_kernel_spmd

    nc = _get_nc()
    hs = np.asarray(hidden_states, dtype=np.float32)
    wg = np.ascontiguousarray(np.asarray(w_gate_up, dtype=np.float32))
    wd = np.ascontiguousarray(np.asarray(w_down, dtype=np.float32))
    in_maps = [
        {
            "xT": np.ascontiguousarray(hs[e].T),
            "wgu": wg[e],
            "wdn": wd[e],
        }
        for e in range(E)
    ]
    res = run_bass_kernel_spmd(nc, in_maps, list(range(E)), trace=trace)
    output = np.stack([res.results[e]["out"] for e in range(E)], axis=0)
    return output, res


def kernel(hidden_states, w_gate_up, w_down):
    output, _ = _run(hidden_states, w_gate_up, w_down, trace=False)
    return output
